# revision 2
# baseline (speedup 1.0000x reference)
"""Trainium2 Bass kernel v2 for nn_ArchGVAE — fp8 DoubleRow edition.

Key structure (vs the 352us fp32r baseline):
- All matmuls fp8e4 (TRN FP8_EXP4). Layer-1/2 messages are DoubleRow pairs:
  u_j = kwd@h_dst + kws@h_src runs as one 256-contract pass; for src-0
  edges the second plane is the host-folded [x0; ea_j] input (chain@kws in
  rows 0-3, kwe in rows 4-8, zero rows elsewhere), so each message is a
  single DR pair; the kwe@ea terms for the other edges are K=5 matmuls
  packed 3-at-a-time into 32-row strips of the PE array (tile_position).
- Layer-0 messages (K=13) and residuals (K=4) strip-packed the same way.
- h lives as fp8 in a 7C panel [h1|h2|h3|p0|p1|p3|x0p] per layer so the
  DR moving pairs are strided [K,2,N] views of one tile.
- leaky_relu fused into custom DVE ops (LEAKY2 = leaky(a)+leaky(b),
  LEAKY_ADD = leaky(a)+b); aggregation split across ACT/DVE/Pool.
- CE without max subtraction (|logits| small): CE = sum ln(sum exp) - sum
  pick; pick accumulated by TENSOR_TENSOR_REDUCE's accumulator; all lns
  deferred to one end pass. CE slots are 15 wide: in4|out4|et5|pad2.
"""
import sys
import math

for _p in ("/opt/trn_rl_repo",):
    if _p not in sys.path:
        sys.path.insert(0, _p)

import numpy as np
import ml_dtypes

import concourse.bass as bass
import concourse.tile as tile
from concourse import bacc, mybir
from concourse import bass_utils
from concourse.dve_ops import (DveOp, DveOpSpec, OPS, CUSTOM_DVE_SPECS,
                               _SUB_OPCODE_FOR_NAME, _CUSTOM_DVE_ROW_BASE,
                               TENSOR_TENSOR_REDUCE, has_src1)
from concourse.dve_spec import Spec, Src0, Src1, C0, Zero, maxx, minn, lower

F32 = mybir.dt.float32
F8 = mybir.dt.float8e4
BF16 = mybir.dt.bfloat16
NPF8 = ml_dtypes.float8_e4m3
NPBF16 = ml_dtypes.bfloat16
AF = mybir.ActivationFunctionType
AX = mybir.AxisListType
DR = mybir.MatmulPerfMode.DoubleRow

B, NODE, ENUM = 65536, 4, 6
XDIM, EDIM, HDIM, ZDIM = 4, 5, 128, 32
SRC = (0, 0, 1, 0, 1, 2)
DST = (1, 2, 2, 3, 3, 3)
NCORE = 8
G = B // NCORE
C = 512
NCH = G // C
NB = C // 128              # graph blocks per chunk (4)
SLOT = 15                  # CE slot: in4|out4|et5|pad2
ALPHA = 0.01
EPS_SCALE = 0.01
BETA = 0.005


# ---------------------------------------------------------------------------
# custom DVE ops
# ---------------------------------------------------------------------------
def _leaky_np(x, a):
    x = np.asarray(x, np.float32)
    return np.maximum(np.nan_to_num(x, nan=0.0), 0) + np.minimum(x, 0) * a


def _register(name, spec):
    for op in OPS:
        if op.name == name:
            return op
    shas = {}
    for ver in ("v3", "v4"):
        r = DveOpSpec(name=name, opcode=0, uops=lower(spec, ver=ver),
                      rd1_en=has_src1(spec))
        shas[ver] = r.sha(ver)
    op = DveOp(name, spec, subdim=False, uops_sha=shas)
    OPS.append(op)
    CUSTOM_DVE_SPECS[name] = spec
    _SUB_OPCODE_FOR_NAME[name] = _CUSTOM_DVE_ROW_BASE + len(OPS) - 1
    assert _SUB_OPCODE_FOR_NAME[name] < 0x20
    return op


# leaky(x) = max(x, a*x) exactly, for 0 < a < 1
LEAKY_ADD = _register(
    "LEAKY_ADD_ANT",
    Spec(
        body=maxx(Src0, Src0 * C0) + Src1,
        reference=lambda in0, in1, s0, s1, imm2: _leaky_np(in0, s0)
        + np.asarray(in1, np.float32),
    ),
)

LEAKY2 = _register(
    "LEAKY2_ANT",
    Spec(
        body=maxx(Src0, Src0 * C0) + maxx(Src1, Src1 * C0),
        reference=lambda in0, in1, s0, s1, imm2: _leaky_np(in0, s0)
        + _leaky_np(in1, s0),
    ),
)

WDEFS = {
    "l0w": (128, 256, F8),
    "wsd1": (128, 2 * HDIM, F8), "wdf1": (128, 2 * HDIM, F8),
    "wsd2": (128, 2 * HDIM, F8), "wdf2": (128, 2 * HDIM, F8),
    "eaw1": (128, HDIM, F8), "eaw2": (128, HDIM, F8),
    "wres1": (HDIM, HDIM, F8), "wres2": (HDIM, HDIM, F8),
    "fc34a": (128, 2 * 64, F8), "fc34b": (128, 2 * 64, F8),
    "fc5": (ZDIM, HDIM, F8),
    "fc6a": (128, 2 * HDIM, F8), "fc6b": (128, 2 * HDIM, F8),
    "d1": (HDIM, 2 * HDIM, F8),
    "d2": (HDIM, 2 * 6 * SLOT, F8),
}


import os
KO = set(os.environ.get("K2_KO", "").split(","))


def build(g=G, nch=NCH, c=C):
    nb = c // 128
    cew = nb * ENUM * SLOT      # CE panel width per chunk
    gw = 3 * ENUM * nb          # sexp groups per chunk
    n = c // 2                  # DR instruction output width

    nc = bacc.Bacc("TRN2", target_bir_lowering=False, debug=False,
                   enable_asserts=False, num_devices=NCORE)

    d_l0 = nc.dram_tensor("l0in", (128, 3 * g), F8, kind="ExternalInput").ap()
    d_ea = nc.dram_tensor("ea245", (128, g), F8, kind="ExternalInput").ap()
    d_pp = {j: nc.dram_tensor(f"p{j}", (XDIM + EDIM, g), F8,
                              kind="ExternalInput").ap() for j in (0, 1, 3)}
    d_x0 = nc.dram_tensor("x0p", (XDIM, g), F8, kind="ExternalInput").ap()
    d_mk = nc.dram_tensor("maskp", (128, (g // 128) * ENUM * SLOT), BF16,
                          kind="ExternalInput").ap()
    d_ep = nc.dram_tensor("epst", (ZDIM, g), BF16, kind="ExternalInput").ap()
    d_w = {k: nc.dram_tensor(k, s[:-1], s[-1], kind="ExternalInput").ap()
           for k, s in WDEFS.items()}
    d_out = nc.dram_tensor("out", (128, 8), F32, kind="ExternalOutput").ap()

    with tile.TileContext(nc) as tc:
        with (
            tc.tile_pool(name="wts", bufs=1) as pw,
            tc.tile_pool(name="acc", bufs=1) as pacc,
            tc.tile_pool(name="pin", bufs=3) as pin,
            tc.tile_pool(name="msb", bufs=3) as pms,
            tc.tile_pool(name="dec", bufs=3) as pdec,
            tc.tile_pool(name="pp", bufs=3, space="PSUM") as pp,  # 2-bank slots
            tc.tile_pool(name="ph", bufs=1, space="PSUM") as ph,  # head psum
        ):
            # ---- persistent weights ----
            w = {}
            for k, shape in WDEFS.items():
                w[k] = pw.tile(list(shape[:-1]), shape[-1], name=f"w_{k}")
                nc.sync.dma_start(w[k][:], d_w[k])
            lneps = pw.tile([ZDIM, 1], F32, name="lneps")
            nc.gpsimd.memset(lneps[:], float(math.log(EPS_SCALE)))

            wsd = {L: w[f"wsd{L}"][:].rearrange("p (two m) -> p two m", two=2)
                   for L in (1, 2)}
            wdf = {L: w[f"wdf{L}"][:].rearrange("p (two m) -> p two m", two=2)
                   for L in (1, 2)}
            fc34a = w["fc34a"][:].rearrange("p (two m) -> p two m", two=2)
            fc34b = w["fc34b"][:].rearrange("p (two m) -> p two m", two=2)
            fc6a = w["fc6a"][:].rearrange("p (two m) -> p two m", two=2)
            fc6b = w["fc6b"][:].rearrange("p (two m) -> p two m", two=2)

            # ---- persistent accumulators ----
            sexp_all = pacc.tile([128, gw * nch], BF16, name="sexp_all")
            acc_pick = pacc.tile([128, 1], F32, name="acc_pick")
            acc_kld = pacc.tile([ZDIM, 3 * nch], F32, name="acc_kld")
            ot = pacc.tile([128, 8], F32, name="ot")
            nc.vector.memset(ot[:], 0.0)
            nc.vector.memset(acc_pick[:], 0.0)

            # ---- persistent fp8 h panels, 4-way rotation across chunks ----
            HP = 8 * c   # pair views may span to o1+2*(o2-o1) <= 8c
            NHB = 4
            hs = pacc.tile([128, NHB * HP], F8, name="hpanels")
            for bf in range(NHB):
                nc.gpsimd.memset(hs[:, bf * HP + 3 * c:(bf + 1) * HP], 0.0)
            # persistent pred panels (2 bufs); slot = in4|P|out4|P|et5 with
            # permanent NEG pads at cols 4 and 9 so exp(pad)=0 and one
            # uniform 5-wide sexp reduce covers all three label groups
            predt = pacc.tile([128, 2 * cew], BF16, name="predt")
            p5 = predt[:].rearrange("p (s i) -> p s i", i=5)
            nc.gpsimd.memset(predt[:], -30000.0)

            for ci in range(nch):
                cs = slice(ci * c, (ci + 1) * c)
                bA, bB, bC = (3 * ci) % NHB, (3 * ci + 1) % NHB, (3 * ci + 2) % NHB
                hA = hs[:, bA * HP:(bA + 1) * HP]
                hB = hs[:, bB * HP:(bB + 1) * HP]
                hC = hs[:, bC * HP:(bC + 1) * HP]

                # ---------------- input DMA ----------------
                l0t = pin.tile([128, 3 * c], F8, name=f"l0_{ci}", tag="l0")
                nc.sync.dma_start(
                    l0t[:].rearrange("p (k x) -> p k x", k=3),
                    d_l0[:].rearrange("p (k x) -> p k x", k=3)[:, :, cs])
                eat = pin.tile([128, c], F8, name=f"ea_{ci}", tag="ea")
                nc.sync.dma_start(eat[:], d_ea[:, cs])
                for jj, off in ((0, 3 * c), (1, 4 * c), (3, 5 * c)):
                    nc.sync.dma_start(hA[0:9, off:off + c], d_pp[jj][:, cs])
                    nc.sync.dma_start(hB[0:9, off:off + c], d_pp[jj][:, cs])
                nc.sync.dma_start(hC[0:XDIM, 6 * c:7 * c], d_x0[:, cs])
                pairw = 2 if nch % 2 == 0 else 1
                if ci % pairw == 0:
                    mk_t = pin.tile([128, pairw * cew], BF16, name=f"mk_{ci}",
                                    tag="mk")
                    nc.sync.dma_start(
                        mk_t[:], d_mk[:, ci * cew:(ci + pairw) * cew])
                ep_t = pin.tile([ZDIM, c], BF16, name=f"ep_{ci}", tag="ep")
                nc.sync.dma_start(ep_t[:], d_ep[:, cs])

                # ---------------- conv layers ----------------
                for L in range(3):
                    hin = (None, hA, hB)[L]
                    hout = (hA, hB, hC)[L]
                    # PSUM slots: T1=[e0|e1] T2=[e2|e3] T3=[e4|e5]
                    T1 = pp.tile([128, 2 * c], F32, name=f"T1_{L}_{ci}", tag="pp")
                    T2 = pp.tile([128, 2 * c], F32, name=f"T2_{L}_{ci}", tag="pp")
                    T3 = pp.tile([128, 2 * c], F32, name=f"T3_{L}_{ci}", tag="pp")
                    msl = [T1[:, 0:c], T1[:, c:2 * c], T2[:, 0:c],
                           T2[:, c:2 * c], T3[:, 0:c], T3[:, c:2 * c]]

                    if L == 0:
                        for j in range(ENUM):
                            blk, st = divmod(j, 4)
                            sp = 32 * st
                            nc.tensor.matmul(
                                msl[j], w["l0w"][sp:sp + 13, 0:128],
                                l0t[sp:sp + 13, blk * c:(blk + 1) * c],
                                start=True, stop=True, tile_position=(sp, 0))
                    else:
                        def pair(o1, o2):
                            d = o2 - o1
                            vw = hin[:, o1:o1 + 2 * d].rearrange(
                                "p (two x) -> p two x", two=2)
                            return vw if d == c else vw[:, :, 0:c]
                        # e2=(h1,h2) e4=(h1,h3) e5=(h2,h3), weights (kws,kwd)
                        # each half's group must close (ea stop) before the
                        # other half starts in the same PSUM bank
                        e245 = ((2, (0, c)), (4, (0, 2 * c)), (5, (c, 2 * c)))
                        for hf in range(2):
                            for j, (o1, o2) in e245:
                                pv = pair(o1, o2)
                                nc.tensor.matmul(
                                    msl[j][:, hf * n:(hf + 1) * n], wsd[L],
                                    pv[:, :, hf * n:(hf + 1) * n],
                                    start=True, stop=False, perf_mode=DR)
                            for i, j in enumerate((2, 4, 5)):
                                sp = 32 * i
                                nc.tensor.matmul(
                                    msl[j][:, hf * n:(hf + 1) * n],
                                    w[f"eaw{L}"][sp:sp + EDIM, :],
                                    eat[sp:sp + EDIM, hf * n:(hf + 1) * n],
                                    start=False, stop=True,
                                    tile_position=(sp, 0))
                        # e0=(h1,p0) e1=(h2,p1) e3=(h3,p3), weights (kwd,fold)
                        for j, (o1, o2) in ((0, (0, 3 * c)), (1, (c, 4 * c)),
                                            (3, (2 * c, 5 * c))):
                            pv = pair(o1, o2)
                            for hf in range(2):
                                nc.tensor.matmul(
                                    msl[j][:, hf * n:(hf + 1) * n], wdf[L],
                                    pv[:, :, hf * n:(hf + 1) * n],
                                    start=True, stop=True, perf_mode=DR)

                    # residuals: T4=[rr2|rr3], T5=[rr1|-]
                    T4 = pp.tile([128, 2 * c], F32, name=f"T4_{L}_{ci}", tag="pp")
                    T5 = pp.tile([128, 2 * c], F32, name=f"T5_{L}_{ci}", tag="pp")
                    rrs = [T5[:, 0:c], T4[:, 0:c], T4[:, c:2 * c]]
                    if L == 0:
                        for i, node in enumerate((1, 2, 3)):
                            blk, st = divmod(6 + i, 4)
                            sp = 32 * st
                            nc.tensor.matmul(
                                rrs[i], w["l0w"][sp:sp + 4, 128:256],
                                l0t[sp:sp + 4, blk * c:(blk + 1) * c],
                                start=True, stop=True, tile_position=(sp, 0))
                    else:
                        for i, node in enumerate((1, 2, 3)):
                            nc.tensor.matmul(
                                rrs[i], w[f"wres{L}"][:],
                                hin[:, i * c:(i + 1) * c],
                                start=True, stop=True)

                    # ---- aggregate (DVE may read at most 1 PSUM input) ----
                    mAs = pms.tile([128, 4 * c], BF16, name=f"mAs{L}_{ci}",
                                   tag="ms")
                    if "prelu" not in KO:
                        nc.scalar.activation(mAs[:, 0:2 * c], T1[:], AF.Prelu,
                                             alpha=ALPHA)
                        nc.scalar.activation(mAs[:, 2 * c:4 * c], T2[:],
                                             AF.Prelu, alpha=ALPHA)
                    t23 = pms.tile([128, 2 * c], BF16, name=f"t23{L}_{ci}",
                                   tag="t23")
                    # t34 = leaky(e4) + e3s ; t345 = leaky(e5) + t34
                    if "t34" not in KO:
                        nc.vector._custom_dve(LEAKY_ADD, out=t23[:, c:2 * c],
                                              in0=T3[:, 0:c],
                                              in1=mAs[:, 3 * c:4 * c],
                                              s0=ALPHA)
                        nc.vector._custom_dve(LEAKY_ADD, out=t23[:, c:2 * c],
                                              in0=T3[:, c:2 * c],
                                              in1=t23[:, c:2 * c], s0=ALPHA)
                    # t12 = e1s + e2s (SBUF-only -> Pool)
                    if "t12" not in KO:
                        nc.gpsimd.tensor_add(t23[:, 0:c], mAs[:, c:2 * c],
                                             mAs[:, 2 * c:3 * c])
                    # finals: h1 then [h2|h3] merged
                    if "fin" not in KO:
                        nc.vector.tensor_add(hout[:, 0:c], mAs[:, 0:c],
                                             T5[:, 0:c])
                        nc.vector.tensor_add(hout[:, c:3 * c], t23[:], T4[:])

                # ---------------- VAE head ----------------
                Tm = ph.tile([128, 2 * c], F32, name=f"Tm_{ci}", tag="ph")
                muv = Tm[0:64, 0:c]
                pab = hC[:, 0:2 * c].rearrange("p (two x) -> p two x", two=2)
                pcd = hC[:, 2 * c:6 * c].rearrange(
                    "p (two x) -> p two x", two=2)[:, :, 0:c]
                for hf in range(2):
                    nc.tensor.matmul(muv[:, hf * n:(hf + 1) * n], fc34a,
                                     pab[:, :, hf * n:(hf + 1) * n],
                                     start=True, stop=False, perf_mode=DR)
                    nc.tensor.matmul(muv[:, hf * n:(hf + 1) * n], fc34b,
                                     pcd[:, :, hf * n:(hf + 1) * n],
                                     start=False, stop=True, perf_mode=DR)
                mu, lv = Tm[0:ZDIM, 0:c], Tm[ZDIM:64, 0:c]
                sfac = pdec.tile([ZDIM, c], F32, name=f"sf_{ci}", tag="sf")
                nc.scalar.activation(sfac[:], lv, AF.Exp, scale=0.5,
                                     bias=lneps[:])
                sq = pdec.tile([ZDIM, c], F32, name=f"sq_{ci}", tag="sq")
                nc.scalar.activation(sq[:], mu, AF.Square,
                                     accum_out=acc_kld[:, ci:ci + 1])
                nc.scalar.activation(sq[:], lv, AF.Exp,
                                     accum_out=acc_kld[:, nch + ci:nch + ci + 1])
                nc.vector.reduce_sum(acc_kld[:, 2 * nch + ci:2 * nch + ci + 1],
                                     lv, axis=AX.X)
                ztf = pdec.tile([ZDIM, c], F32, name=f"ztf_{ci}", tag="ztf")
                nc.gpsimd.tensor_mul(ztf[:], ep_t[:], sfac[:])
                zt = pdec.tile([ZDIM, c], F8, name=f"zt_{ci}", tag="zt")
                nc.vector.tensor_add(zt[:], ztf[:], mu)

                Th = ph.tile([128, 2 * c], F32, name=f"Th_{ci}", tag="ph")
                nc.tensor.matmul(Th[:, 0:c], w["fc5"][:], zt[:], start=True,
                                 stop=True)
                Hg = pdec.tile([128, c], F8, name=f"Hg_{ci}", tag="Hg")
                nc.scalar.activation(Hg[:], Th[:, 0:c], AF.Tanh)

                Td = ph.tile([128, 2 * c], F32, name=f"Td_{ci}", tag="ph")
                nc.tensor.matmul(Td[:, 0:c], w["d1"][:, 0:HDIM], Hg[:],
                                 start=True, stop=True)
                nc.tensor.matmul(Td[:, c:2 * c], w["d1"][:, HDIM:2 * HDIM],
                                 Hg[:], start=True, stop=True)
                ta = pdec.tile([128, c], BF16, name=f"ta_{ci}", tag="ta")
                nc.scalar.activation(ta[:], Td[:, 0:c], AF.Prelu, alpha=ALPHA)
                h1d = pdec.tile([128, c], F8, name=f"h1d_{ci}", tag="h1d")
                nc.vector.tensor_add(h1d[:], ta[:], Td[:, c:2 * c])

                # d2 role-swap: mw panel -> T6 bank0, rw panel -> bank1
                T6 = ph.tile([128, 2 * c], F32, name=f"T6_{ci}", tag="ph")
                for k in range(nb):
                    hblk = h1d[:, 128 * k:128 * (k + 1)]
                    nc.tensor.matmul(T6[:, k * 90:(k + 1) * 90], hblk,
                                     w["d2"][:, 0:90], start=True, stop=True)
                    nc.tensor.matmul(T6[:, 512 + k * 90:512 + (k + 1) * 90],
                                     hblk, w["d2"][:, 90:180],
                                     start=True, stop=True)
                mws = pdec.tile([128, cew], BF16, name=f"mws_{ci}", tag="mws")
                nc.scalar.activation(mws[:], T6[:, 0:cew], AF.Prelu,
                                     alpha=ALPHA)
                prd = predt[:, (ci % 2) * cew:(ci % 2 + 1) * cew]
                prs = prd.rearrange("p (s i) -> p s i", i=SLOT)
                msv = mws[:].rearrange("p (s i) -> p s i", i=SLOT)
                rwv = T6[:, 512:512 + cew].rearrange("p (s i) -> p s i", i=SLOT)
                for lo, hi in ((0, 4), (5, 9), (10, 15)):
                    nc.vector.tensor_add(prs[:, :, lo:hi], msv[:, :, lo:hi],
                                         rwv[:, :, lo:hi])

                # ---------------- CE (batched per chunk pair) ----------------
                if ci % pairw == pairw - 1:
                    pboth = predt[:, 0:pairw * cew]
                    eb = pdec.tile([128, pairw * cew], BF16, name=f"eb_{ci}",
                                   tag="eb")
                    nc.scalar.activation(eb[:], pboth, AF.Exp)
                    e5 = eb[:].rearrange("p (s i) -> p s i", i=5)
                    so = (ci - pairw + 1) * gw
                    with nc.allow_low_precision(reason="bf16 sexp, ln later"):
                        nc.vector.reduce_sum(sexp_all[:, so:so + pairw * gw],
                                             e5, axis=AX.X)
                    junk = pdec.tile([128, pairw * cew], BF16,
                                     name=f"junk_{ci}", tag="junk")
                    nc.vector._custom_dve(
                        TENSOR_TENSOR_REDUCE, out=junk[:], in0=mk_t[:],
                        in1=pboth, s0=acc_pick[:, 0:1], s1=1.0,
                        accum_out=acc_pick[:, 0:1])

            # ---- final: deferred ln + KLD reduction ----
            lnb = pacc.tile([128, gw * nch], F32, name="lnb")
            nc.scalar.activation(lnb[:], sexp_all[:], AF.Ln,
                                 accum_out=ot[:, 0:1])
            nc.vector.tensor_copy(ot[:, 1:2], acc_pick[:])
            nc.vector.reduce_sum(ot[0:ZDIM, 2:3], acc_kld[:, 0:nch], axis=AX.X)
            nc.vector.reduce_sum(ot[0:ZDIM, 3:4], acc_kld[:, nch:2 * nch],
                                 axis=AX.X)
            nc.vector.reduce_sum(ot[0:ZDIM, 4:5], acc_kld[:, 2 * nch:3 * nch],
                                 axis=AX.X)
            nc.sync.dma_start(d_out, ot[:])

    nc.compile()
    return nc


# ---------------------------------------------------------------------------
# host packing
# ---------------------------------------------------------------------------
def _f8(x):
    return np.asarray(x, np.float32).astype(NPF8)


def _pack_host(inputs, g=G, nch=NCH, c=C):
    f32 = np.float32
    x = np.ascontiguousarray(inputs["x"], dtype=f32).reshape(NCORE, g, NODE, XDIM)
    ea = np.ascontiguousarray(inputs["edge_attr"], dtype=f32).reshape(
        NCORE, g, ENUM, EDIM)
    arch = np.ascontiguousarray(inputs["arch_tensor"], dtype=f32).reshape(
        NCORE, g, ENUM, 13)
    eps = np.ascontiguousarray(inputs["eps"], dtype=f32).reshape(NCORE, g, ZDIM)

    for bname in ("c0_rb1", "c0_rb2", "c1_rb1", "c1_rb2", "c2_rb1", "c2_rb2",
                  "fc3_b", "fc4_b", "fc5_b", "d1_mb", "d1_rb", "d2_mb", "d2_rb"):
        assert not np.any(np.asarray(inputs[bname])), f"nonzero bias {bname}"

    def W(k):
        return np.asarray(inputs[k], np.float64)

    W0 = W("c0_rw1") @ W("c0_rw2")
    W1 = W("c1_rw1") @ W("c1_rw2")
    W2 = W("c2_rw1") @ W("c2_rw2")
    chain1, chain2, chain3 = W0, W0 @ W1, W0 @ W1 @ W2
    kw0 = np.asarray(inputs["c0_kw"], f32)
    kw1, kw2 = W("c1_kw"), W("c2_kw")
    fc34 = np.concatenate([W("fc3_w"), W("fc4_w")], axis=1)

    x8 = _f8(x)
    ea8 = _f8(ea)
    l0 = np.zeros((NCORE, 128, 3 * g), NPF8)
    for j in range(ENUM):
        blk, st = divmod(j, 4)
        sp = 32 * st
        m0 = np.concatenate([x8[:, :, DST[j]], x8[:, :, SRC[j]],
                             ea8[:, :, j]], axis=2)
        l0[:, sp:sp + 13, blk * g:(blk + 1) * g] = m0.transpose(0, 2, 1)
    for i, node in enumerate((1, 2, 3)):
        blk, st = divmod(6 + i, 4)
        sp = 32 * st
        l0[:, sp:sp + 4, blk * g:(blk + 1) * g] = \
            x8[:, :, node].transpose(0, 2, 1)
    ea245 = np.zeros((NCORE, 128, g), NPF8)
    for i, j in enumerate((2, 4, 5)):
        ea245[:, 32 * i:32 * i + EDIM] = ea8[:, :, j].transpose(0, 2, 1)
    pads = {}
    for j in (0, 1, 3):
        pads[j] = np.ascontiguousarray(np.concatenate(
            [x8[:, :, 0], ea8[:, :, j]], axis=2).transpose(0, 2, 1))
    x0p = np.ascontiguousarray(x8[:, :, 0].transpose(0, 2, 1))

    # CE mask panel, slot layout in4|out4|et5|pad2 (bf16)
    nblocks = g // 128
    mk = np.zeros((NCORE, nblocks, 128, ENUM, SLOT), f32)
    a6 = arch.reshape(NCORE, nblocks, 128, ENUM, 13)
    for off, wd, lo in ((0, 4, 0), (4, 4, 5), (8, 5, 10)):
        blkv = a6[..., off:off + wd]
        mx = blkv.max(axis=-1, keepdims=True)
        mk[..., lo:lo + wd] = (blkv == mx)
    mk = mk.transpose(0, 2, 1, 3, 4).reshape(
        NCORE, 128, nblocks * ENUM * SLOT).astype(NPBF16)

    epst = np.ascontiguousarray(eps.transpose(0, 2, 1)).astype(NPBF16)

    # ---- weights ----
    l0w = np.zeros((128, 256), NPF8)
    kw08 = _f8(kw0)
    W08 = _f8(W0)
    for st in range(4):
        l0w[32 * st:32 * st + 13, 0:128] = kw08
    for st in range(3):
        l0w[32 * st:32 * st + 4, 128:256] = W08

    def drpack(p0_, p1_):
        K, M = p0_.shape
        out = np.zeros((K, 2, M), NPF8)
        out[:, 0] = _f8(p0_)
        out[:, 1] = _f8(p1_)
        return out.reshape(K, 2 * M)

    wts = {"l0w": l0w}
    for L, kw, chain in ((1, kw1, chain1), (2, kw2, chain2)):
        kwd, kws, kwe = kw[0:HDIM], kw[HDIM:2 * HDIM], kw[2 * HDIM:]
        wts[f"wsd{L}"] = drpack(kws, kwd)
        fold = np.zeros((HDIM, HDIM))
        fold[0:XDIM] = chain @ kws
        fold[XDIM:XDIM + EDIM] = kwe
        wts[f"wdf{L}"] = drpack(kwd, fold)
        eaw = np.zeros((128, HDIM), NPF8)
        for i in range(3):
            eaw[32 * i:32 * i + EDIM] = _f8(kwe)
        wts[f"eaw{L}"] = eaw
        wts[f"wres{L}"] = _f8(W1 if L == 1 else W2)
    wts["fc34a"] = drpack(fc34, fc34)
    fold34 = np.zeros((HDIM, 2 * ZDIM))
    fold34[0:XDIM] = chain3 @ fc34
    wts["fc34b"] = drpack(fc34, fold34)
    wts["fc5"] = _f8(np.asarray(inputs["fc5_w"], f32))
    fc6 = fc34[:, 0:ZDIM] @ W("fc5_w")
    wts["fc6a"] = drpack(fc6, fc6)
    fold6 = np.zeros((HDIM, HDIM))
    fold6[0:XDIM] = chain3 @ fc6
    wts["fc6b"] = drpack(fc6, fold6)
    wts["d1"] = _f8(np.concatenate([inputs["d1_mw"], inputs["d1_rw"]], axis=1))
    d2m = np.asarray(inputs["d2_mw"], f32)
    d2r = np.asarray(inputs["d2_rw"], f32)
    d2 = np.zeros((HDIM, 2 * ENUM * SLOT), f32)
    for j in range(ENUM):
        for part, src_np in ((0, d2m), (ENUM * SLOT, d2r)):
            base = part + SLOT * j
            d2[:, base + 0:base + 4] = src_np[:, 13 * j + 0:13 * j + 4]
            d2[:, base + 5:base + 9] = src_np[:, 13 * j + 4:13 * j + 8]
            d2[:, base + 10:base + 15] = src_np[:, 13 * j + 8:13 * j + 13]
    wts["d2"] = _f8(d2)

    in_maps = []
    for core in range(NCORE):
        m = {
            "l0in": np.ascontiguousarray(l0[core]),
            "ea245": np.ascontiguousarray(ea245[core]),
            "p0": np.ascontiguousarray(pads[0][core]),
            "p1": np.ascontiguousarray(pads[1][core]),
            "p3": np.ascontiguousarray(pads[3][core]),
            "x0p": np.ascontiguousarray(x0p[core]),
            "maskp": np.ascontiguousarray(mk[core]),
            "epst": np.ascontiguousarray(epst[core]),
        }
        m.update(wts)
        in_maps.append(m)
    return in_maps


def _combine_host(outs):
    lnsum = pick = mu2 = elv = lvt = 0.0
    for o in outs:
        o = np.asarray(o, np.float64)
        lnsum += o[:, 0].sum()
        pick += o[:, 1].sum()
        mu2 += o[0:ZDIM, 2].sum()
        elv += o[0:ZDIM, 3].sum()
        lvt += o[0:ZDIM, 4].sum()
    res = (lnsum - pick) / (B * ENUM)
    kld_inner = (B * ZDIM) + lvt - mu2 - elv
    kld = -0.5 * kld_inner / (B * ZDIM)
    return np.float32(res + BETA * kld)


_NC_CACHE = {}


def _get_nc():
    if "nc" not in _NC_CACHE:
        _NC_CACHE["nc"] = build()
    return _NC_CACHE["nc"]


def kernel(**inputs):
    nc = _get_nc()
    in_maps = _pack_host(inputs)
    res = bass_utils.run_bass_kernel_spmd(nc, in_maps,
                                          core_ids=list(range(NCORE)))
    outs = [r["out"] for r in res.results]
    return np.array(_combine_host(outs), dtype=np.float32)
